# revision 2
# baseline (speedup 1.0000x reference)
"""Trainium2 Bass kernel for CausalSelfAttention (PentaNet-quantized weights).

Reference computation (B=2, T=2048, C=1024, H=16 heads, D=64):
    qkv = x @ quant(w_attn).T ; split q,k,v ; causal softmax attention ;
    out = y @ quant(w_proj).T

Sharding: 8 cores = 2 (batch) x 4 (head groups of 4 heads).  Each core
computes its batch element's attention for its 4 heads plus the partial
output projection over its 256 input channels; the host sums the 4
partials per batch (the w_proj contraction is split across head groups).

Device layout avoids all on-chip transposes:
  - host supplies xT = x[b].T  [C, T]
  - qT,kT computed as [o, t] (weights stationary), v as [t, o]
  - scores computed transposed: ST[j, i] = k_j . q_i  (j = key pos)
  - P = exp(ST/8) with causal masking (block-skip + triangular mask)
  - OT[d, i] = sum_j V[j, d] P[j, i] accumulated in PSUM; an extra
    ones-column in V yields the softmax denominator as OT row 64
  - OT normalized is exactly the lhsT the projection needs.
All matmuls run in bf16 (fp32 PSUM accumulation).  Paired heads sit at
partitions 0:64 / 64:128 so their K=64 score matmuls row-tile onto
disjoint halves of the PE array and overlap.  Output partials are
stored bf16 and summed on host in fp32.
"""

import os
import sys

sys.path.insert(0, "/opt/trn_rl_repo")

import numpy as np
import ml_dtypes

import jax

try:
    jax.config.update("jax_compilation_cache_dir", "/root/.cache/jax_bass_neff")
except Exception:
    pass

import concourse.bass as bass
import concourse.tile as tile
from concourse import bacc, mybir
from concourse.bass_utils import run_bass_kernel_spmd

F32 = mybir.dt.float32
F32R = mybir.dt.float32r
BF16 = mybir.dt.bfloat16

B, T, C = 2, 2048, 1024
H, D = 16, 64
HL = 4                    # heads per core
OL = HL * D               # 256 local output channels
KT = C // 128             # 8 k-tiles over C
TT = T // 128             # 16 t-tiles
NCH = T // 512            # 4 i-chunks of 512
SCALE = 1.0 / 8.0         # 1/sqrt(D)

PAIRED_ST = True          # row-tile head-pair score matmuls


def r(ap):
    return ap


def build_body(ctx, tc, xT, wq, wk, wv, wp, tri, ztri, tri2, onesd, out):
    nc = tc.nc

    consts = ctx.enter_context(tc.tile_pool(name="consts", bufs=1))
    acts = ctx.enter_context(tc.tile_pool(name="acts", bufs=1))
    pp = ctx.enter_context(tc.tile_pool(name="pp", bufs=4))
    pfp = ctx.enter_context(tc.tile_pool(name="pfp", bufs=4))
    rcp = ctx.enter_context(tc.tile_pool(name="rcp", bufs=2))
    bbp = ctx.enter_context(tc.tile_pool(name="bbp", bufs=2))
    obp = ctx.enter_context(tc.tile_pool(name="obp", bufs=3))
    ps_mm = ctx.enter_context(tc.tile_pool(name="ps_mm", bufs=2, space="PSUM"))
    ps_pj = ctx.enter_context(tc.tile_pool(name="ps_pj", bufs=2, space="PSUM"))
    ps_ot = ctx.enter_context(tc.tile_pool(name="ps_ot", bufs=2, space="PSUM"))

    # ---- load inputs to SBUF ----
    # Big input stream on the SP HWDGE queue: wq and x chunk 0 in halves
    # (so the first qk matmuls start after ~0.5 MB), then wk, wv, the other
    # x chunks, wp.  Small constants go on the gpsimd SWDGE queue; output
    # stores use the Activation HWDGE queue (see proj_unit) to keep all
    # three queues' fixed descriptor-generation costs off each other.
    wq_sb = consts.tile([128, KT * OL], BF16)
    wk_sb = consts.tile([128, KT * OL], BF16)
    wv_sb = consts.tile([128, KT * OL], BF16)
    xT_sb = consts.tile([128, KT * T], BF16)

    def load_w(w_sb, w_d, k0, k1):
        nc.sync.dma_start(
            w_sb[:].rearrange("p (k o) -> p k o", k=KT)[:, k0:k1],
            w_d.rearrange("(k p) o -> p k o", k=KT)[:, k0:k1])

    def load_x(n):
        nc.sync.dma_start(
            xT_sb[:].rearrange("p (k t) -> p k t", k=KT)[:, :, n * 512:(n + 1) * 512],
            xT.rearrange("(k p) t -> p k t", k=KT)[:, :, n * 512:(n + 1) * 512])

    q_sb = acts.tile([128, 2 * T], BF16)
    k_sb = acts.tile([128, 2 * T], BF16)
    v_sb = acts.tile([128, TT * HL * (2 * D)], BF16)
    # per-i-chunk yt tiles: [128, kk*512] layout so proj(ic) carries no
    # dependency on later chunks' normalize writes
    yt_c = [acts.tile([128, 2 * 512], BF16, name=f"yt_c{i}") for i in range(NCH)]

    # main input stream
    def load_x0(k0, k1, eng):
        eng.dma_start(
            xT_sb[:].rearrange("p (k t) -> p k t", k=KT)[:, k0:k1, 0:512],
            xT.rearrange("(k p) t -> p k t", k=KT)[:, k0:k1, 0:512])

    def load_wk(k0, k1):
        nc.scalar.dma_start(
            wk_sb[:].rearrange("p (k o) -> p k o", k=KT)[:, k0:k1],
            wk.rearrange("(k p) o -> p k o", k=KT)[:, k0:k1])

    # SP queue: wq halves, x0 tail, x1, wv, masks, x2, x3, wp
    # scalar queue: x0 head (parallel with wq), then wk halves
    load_w(wq_sb, wq, 0, 2)
    load_x0(0, 2, nc.scalar)
    load_w(wq_sb, wq, 2, 4)
    load_x0(2, 4, nc.scalar)
    load_w(wq_sb, wq, 4, 8)
    load_wk(0, 4)
    load_x0(4, 8, nc.sync)
    load_wk(4, 8)
    load_x(1)
    load_w(wv_sb, wv, 0, 8)
    tri_sb = consts.tile([128, 128], BF16)
    nc.sync.dma_start(tri_sb[:], tri[:, 0:128])
    ztri_sb = consts.tile([128, 256], BF16)
    nc.sync.dma_start(ztri_sb[:], ztri[:, :])
    tri_sb2 = consts.tile([128, 256], BF16)
    nc.sync.dma_start(tri_sb2[:], tri2[:, :])
    for n in range(2, NCH):
        load_x(n)
    wp_sb = consts.tile([128, 2 * C], BF16)
    nc.sync.dma_start(
        wp_sb[:].rearrange("p (k o) -> p k o", k=2),
        wp.rearrange("(k p) o -> p k o", k=2))

    # ones columns (indices D:2D) of every [t-tile, head] V block: the
    # AV matmul replicates the softmax denominator onto partitions 64:128,
    # so normalization needs no partition broadcast.
    v_ones = v_sb[:].rearrange("p (g c) -> p g c", c=2 * D)[:, :, D:]
    nc.gpsimd.memset(v_ones, 1.0)

    # ---- emission units ----
    def qk_unit(n, which, m):
        w_sb, dst = (wq_sb, q_sb) if which == 0 else (wk_sb, k_sb)
        ps = ps_pj.tile([128, 512], F32, tag="pj")
        for k in range(KT):
            nc.tensor.matmul(
                ps[:],
                r(w_sb[:, k * OL + m * 128: k * OL + (m + 1) * 128]),
                r(xT_sb[:, k * T + n * 512: k * T + (n + 1) * 512]),
                start=(k == 0), stop=(k == KT - 1),
            )
        nc.vector.tensor_copy(dst[:, m * T + n * 512: m * T + (n + 1) * 512], ps[:])

    def v_unit(t):
        ps = ps_pj.tile([128, OL], F32, tag="pj")
        for k in range(KT):
            nc.tensor.matmul(
                ps[:],
                r(xT_sb[:, k * T + t * 128: k * T + (t + 1) * 128]),
                r(wv_sb[:, k * OL:(k + 1) * OL]),
                start=(k == 0), stop=(k == KT - 1),
            )
        dst = v_sb[:, t * HL * 2 * D: (t + 1) * HL * 2 * D]
        dst = dst.rearrange("p (h c) -> p h c", h=HL)[:, :, 0:D]
        nc.vector.tensor_copy(dst, ps[:].rearrange("p (h c) -> p h c", h=HL))

    def proj_unit(t, n2, split_store=False):
        ps = ps_pj.tile([128, 512], F32, tag="pj")
        for kk in range(2):
            nc.tensor.matmul(
                ps[:],
                r(yt_c[t // 4][:, kk * 512 + (t % 4) * 128: kk * 512 + (t % 4 + 1) * 128]),
                r(wp_sb[:, kk * C + n2 * 512: kk * C + (n2 + 1) * 512]),
                start=(kk == 0), stop=(kk == 1),
            )
        ob = _ob_state.get(t)
        if ob is None:
            ob = obp.tile([128, 1024], BF16, tag="ob", name=f"ob_{t}")
            _ob_state[t] = ob
        nc.vector.tensor_copy(ob[:, n2 * 512:(n2 + 1) * 512], ps[:])
        q = nc.scalar if (t + n2) % 2 else nc.sync
        if split_store:
            q.dma_start(out[t * 128:(t + 1) * 128, n2 * 512:(n2 + 1) * 512],
                        ob[:, n2 * 512:(n2 + 1) * 512])
            if t in _ob_done:
                del _ob_state[t]
                del _ob_done[t]
            else:
                _ob_done[t] = True
        elif t in _ob_done:
            q.dma_start(out[t * 128:(t + 1) * 128, :], ob[:])
            del _ob_state[t]
            del _ob_done[t]
        else:
            _ob_done[t] = True

    _ob_state = {}
    _ob_done = {}
    _prefilled = {}
    _st_exp_fns = {}

    def qkv_units(n):
        return ([(lambda n=n, w=w, m=m: qk_unit(n, w, m)) for w in range(2) for m in range(2)]
                + [(lambda t=t: v_unit(t)) for t in range(4 * n, 4 * n + 4)])

    def proj_units(ic, split_store=False):
        return [(lambda t=t, n2=n2: proj_unit(t, n2, split_store))
                for t in range(4 * ic, 4 * ic + 4) for n2 in range(2)]

    # ---- attention ----
    # Heads are processed in pairs (pb 0 and 64 share an i-chunk's blocks);
    # with PAIRED_ST the two heads' K=64 score matmuls are emitted
    # adjacently so they row-tile onto disjoint halves of the PE array.
    def attn_blocks(ic):
        items = []
        for hp in range(2):           # head pair: heads (2hp, 2hp+1)
            mo = hp * T
            state = {}

            def ot_mm(hx, tj, rhs, cs, ic=ic, state=state):
                vh = v_sb[:, (tj * HL + 2 * (state["hp"]) + hx) * 2 * D:
                          (tj * HL + 2 * (state["hp"]) + hx + 1) * 2 * D]
                nc.tensor.matmul(
                    state["ps_o"][hx][:, cs:512],
                    r(vh), r(rhs),
                    start=(tj == 0), stop=(tj == 4 * ic + 3),
                    skip_group_check=True,
                )

            def open_ps(ic=ic, state=state, hp=hp):
                state["hp"] = hp
                state["ps_o"] = [
                    ps_ot.tile([128, 512], F32, tag="ot", name=f"ps_o_{ic}_{hp}_{hx}")
                    for hx in range(2)]

            def st_exp(tja, hp, dst_pool, tag, mo=mo, ic=ic):
                # score matmuls + exp for j-tiles (tja, tja+1), both heads
                ps_s = [ps_mm.tile([128, 1024], F32, tag="mm",
                                   name=f"ps_s_{ic}_{hp}_{hx}")
                        for hx in range(2)]
                # interleave the two heads' matmuls: adjacent instructions
                # target row groups 0:64 / 64:128 and overlap on HW
                for j in range(2):
                    for hx in range(2):
                        pb = 64 * hx
                        nc.tensor.matmul(
                            ps_s[hx][:, j * 512:(j + 1) * 512],
                            r(k_sb[pb:pb + 64, mo + (tja + j) * 128: mo + (tja + j + 1) * 128]),
                            r(q_sb[pb:pb + 64, mo + ic * 512: mo + (ic + 1) * 512]),
                            start=True, stop=True,
                            skip_group_check=True,
                        )
                p_t = [dst_pool.tile([128, 1024], BF16, tag=tag,
                                     name=f"p_t_{ic}_{hp}_{hx}") for hx in range(2)]
                for hx in range(2):
                    nc.scalar.activation(p_t[hx][:], ps_s[hx][:],
                                         mybir.ActivationFunctionType.Exp, scale=SCALE)
                return p_t

            def full_pair(tja, mo=mo, ic=ic, state=state, open_ps=open_ps,
                          ot_mm=ot_mm, st_exp=st_exp, hp=hp):
                if tja == 0:
                    open_ps()
                p_t = _prefilled.pop((ic, hp, tja), None)
                if p_t is None:
                    p_t = st_exp(tja, hp, pp, "p")
                for hx in range(2):
                    for j in range(2):
                        ot_mm(hx, tja + j, p_t[hx][:, j * 512:(j + 1) * 512], 0)

            def diag_pair(da, mo=mo, ic=ic, state=state, open_ps=open_ps, ot_mm=ot_mm):
                # da=0: blocks d=0 (w 512, cs 0) + d=1 (w 384, cs 128) in one
                # [128, 896] tile/exp.  da=2: d=2 + d=3 (both w 256, cs 256)
                # in one [128, 512] tile/exp.  d=3 uses the zero|tri mask.
                if 4 * ic + da == 0:
                    open_ps()
                if da == 0:
                    widths, css = (512, 384), (0, 128)
                else:
                    widths, css = (256, 128), (256, 384)
                tot_w = widths[0] + widths[1]
                ps_s = [ps_mm.tile([128, tot_w], F32, tag="mm",
                                   name=f"ps_d_{ic}_{state['hp']}_{hx}")
                        for hx in range(2)]
                for j in range(2):
                    d = da + j
                    tj = 4 * ic + d
                    off = 0 if j == 0 else widths[0]
                    for hx in range(2):
                        pb = 64 * hx
                        nc.tensor.matmul(
                            ps_s[hx][:, off:off + widths[j]],
                            r(k_sb[pb:pb + 64, mo + tj * 128: mo + (tj + 1) * 128]),
                            r(q_sb[pb:pb + 64, mo + ic * 512 + css[j]: mo + (ic + 1) * 512]),
                            start=True, stop=True,
                            skip_group_check=True,
                        )
                p_t = [pp.tile([128, tot_w], BF16, tag="p",
                               name=f"p_d_{ic}_{state['hp']}_{hx}") for hx in range(2)]
                for hx in range(2):
                    nc.scalar.activation(p_t[hx][:], ps_s[hx][:],
                                         mybir.ActivationFunctionType.Exp, scale=SCALE)
                for hx in range(2):
                    if da == 0:
                        nc.vector.tensor_mul(p_t[hx][:, 0:128], p_t[hx][:, 0:128], tri_sb[:])
                        nc.vector.tensor_mul(p_t[hx][:, 512:640], p_t[hx][:, 512:640], tri_sb[:])
                    else:
                        nc.vector.tensor_mul(p_t[hx][:, 0:256], p_t[hx][:, 0:256], tri_sb2[:])
                        nc.vector.tensor_mul(p_t[hx][:, 256:384], p_t[hx][:, 256:384], tri_sb[:])
                for hx in range(2):
                    off = 0
                    for j in range(2):
                        ot_mm(hx, 4 * ic + da + j, p_t[hx][:, off:off + widths[j]], css[j])
                        off += widths[j]
                    if da == 2:
                        # normalize this head while the other head's AV
                        # matmuls run: yt[o, i] = OT[d, i] / s[i]; the
                        # denominator sits replicated on ps_o rows 64:128.
                        pb = 64 * hx
                        ps_o = state["ps_o"][hx]
                        rc = rcp.tile([64, 512], F32R, tag="rc", name=f"rc_{hx}")
                        with tc.high_priority():
                            with nc.allow_low_precision(reason="fp32r ~ fp32 denom"):
                                nc.vector.reciprocal(rc[:], ps_o[D:2 * D, :])
                            kk = mo // T
                            nc.vector.tensor_mul(
                                yt_c[ic][pb:pb + 64, kk * 512:(kk + 1) * 512],
                                ps_o[0:D, :], rc[:],
                            )

            _st_exp_fns[(ic, hp)] = st_exp
            for tja in range(0, 4 * ic, 2):
                items.append(lambda tja=tja, f=full_pair: f(tja))
            for da in (0, 2):
                items.append(lambda da=da, f=diag_pair: f(da))
        return items

    def prefill_units(ic, tjas):
        def mk(tja, hp):
            def go():
                _prefilled[(ic, hp, tja)] = _st_exp_fns[(ic, hp)](
                    tja, hp, pfp, "pf")
            return go
        return [mk(tja, hp) for tja in tjas for hp in range(2)]

    def emit_interleaved(blocks, fillers, weights=None):
        """Emit attention blocks with fillers spread by weight between."""
        nf = len(fillers)
        if weights is None:
            weights = [1] * len(blocks)
        tot = sum(weights)
        fi = 0
        acc = 0
        for blk, w in zip(blocks, weights):
            blk()
            acc += w
            want = acc * nf // tot
            while fi < want:
                fillers[fi]()
                fi += 1
        while fi < nf:
            fillers[fi]()
            fi += 1

    # schedule: qkv(0) first; attention chunk ic interleaves qkv(ic+1) and
    # proj(ic-1); proj(3) trails.
    for u in qkv_units(0):
        u()
    # Fillers are balanced against each chunk's exp (ACT) load, which grows
    # with ic: chunk1 <- qkv(2)+proj(0); chunk2 <- qkv(3) minus its last two
    # v-units; chunk3 <- those v-units + proj(1) + proj(2).
    all_blocks = [attn_blocks(ic) for ic in range(NCH)]
    fills = {
        0: qkv_units(1),
        1: qkv_units(2) + proj_units(0),
        2: qkv_units(3)[:-2],
        3: qkv_units(3)[-2:] + proj_units(1) + proj_units(2),
    }
    for ic in range(NCH):
        emit_interleaved(all_blocks[ic], fills[ic])
    for u in proj_units(NCH - 1, split_store=True):
        u()


def build_program(reps=1):
    from contextlib import ExitStack

    nc = bacc.Bacc("TRN2", target_bir_lowering=False, debug=False)
    xT = nc.dram_tensor("xT", [C, T], BF16, kind="ExternalInput").ap()
    wq = nc.dram_tensor("wq", [C, OL], BF16, kind="ExternalInput").ap()
    wk = nc.dram_tensor("wk", [C, OL], BF16, kind="ExternalInput").ap()
    wv = nc.dram_tensor("wv", [C, OL], BF16, kind="ExternalInput").ap()
    wp = nc.dram_tensor("wp", [OL, C], BF16, kind="ExternalInput").ap()
    tri = nc.dram_tensor("tri", [128, 128], BF16, kind="ExternalInput").ap()
    ztri = nc.dram_tensor("ztri", [128, 256], BF16, kind="ExternalInput").ap()
    tri2 = nc.dram_tensor("tri2", [128, 256], BF16, kind="ExternalInput").ap()
    onesd = nc.dram_tensor("onesd", [128, TT * HL], BF16, kind="ExternalInput").ap()
    out = nc.dram_tensor("out", [T, C], BF16, kind="ExternalOutput").ap()

    with tile.TileContext(nc) as tc:
        for _ in range(reps):
            with ExitStack() as ctx:
                build_body(ctx, tc, xT, wq, wk, wv, wp, tri, ztri, tri2, onesd, out)
    nc.compile()
    return nc


def quant_weight_np(w):
    scale = max(np.mean(np.abs(w), dtype=np.float32), np.float32(1e-8))
    return (np.clip(np.round(w / scale), -2.0, 2.0) * scale).astype(np.float32)


def make_in_maps(x, w_attn, w_proj):
    wq_f = quant_weight_np(w_attn)
    wp_f = quant_weight_np(w_proj)
    tri = np.triu(np.ones((128, 128), dtype=np.float32))
    ztri = np.concatenate([np.zeros((128, 128), dtype=np.float32), tri], axis=1)
    tri2 = np.concatenate([tri, np.ones((128, 128), dtype=np.float32)], axis=1)
    in_maps = []
    for core in range(8):
        b, g = divmod(core, 4)
        sl = slice(g * OL, (g + 1) * OL)
        in_maps.append({
            "xT": np.ascontiguousarray(x[b].T).astype(ml_dtypes.bfloat16),
            "wq": np.ascontiguousarray(wq_f[0 * C:1 * C][sl].T).astype(ml_dtypes.bfloat16),
            "wk": np.ascontiguousarray(wq_f[1 * C:2 * C][sl].T).astype(ml_dtypes.bfloat16),
            "wv": np.ascontiguousarray(wq_f[2 * C:3 * C][sl].T).astype(ml_dtypes.bfloat16),
            "wp": np.ascontiguousarray(wp_f[:, sl].T).astype(ml_dtypes.bfloat16),
            "tri": tri.astype(ml_dtypes.bfloat16),
            "ztri": ztri.astype(ml_dtypes.bfloat16),
            "tri2": tri2.astype(ml_dtypes.bfloat16),
            "onesd": np.ones((128, TT * HL), dtype=ml_dtypes.bfloat16),
        })
    return in_maps


_CACHED_NC = None


def kernel(x, w_attn, w_proj):
    global _CACHED_NC
    if _CACHED_NC is None:
        _CACHED_NC = build_program()
    in_maps = make_in_maps(np.asarray(x, dtype=np.float32),
                           np.asarray(w_attn, dtype=np.float32),
                           np.asarray(w_proj, dtype=np.float32))
    res = run_bass_kernel_spmd(_CACHED_NC, in_maps, list(range(8)))
    out = np.zeros((B, T, C), dtype=np.float32)
    for core in range(8):
        b = core // 4
        out[b] += np.asarray(res.results[core]["out"], dtype=np.float32)
    return out
